# revision 3
# baseline (speedup 1.0000x reference)
"""Trainium2 Bass kernel for nn_CharmGNN (2-step GNN message passing).

Sharding: 8 cores = 4 batches x 2 row-halves. Each core owns half the
p-rows (1024), q-rows (512), qu-rows (16) of one batch element.

Key algebra:
  - qp / qu_p / qu_q "adjacencies" are rank-1 (outer products of masks):
    their matmuls collapse to weighted reductions u[d] = sum_j w_j node[j,d]
    followed by tiny matvecs v = u @ W.
  - per-row scale s_i = mask_i / nb_i is folded into the transposed graph
    (scaled while rows are still on partitions, before the transpose).
  - source weights w_j = pm_j * sigmoid(z_j + b_node) are computed as
    sigmoid(z_j + b_node + (pm_j - 1) * 30)   (sigmoid(-30) ~ 1e-13 ~ 0).
  - all updates run feature-major (FM: [D on partitions, tokens free]);
    token-major (TM) copies are only needed by the u-reductions and come
    from a DMA-transpose of the all-gathered DRAM buffer.

SPMD cleanliness: the NEFF is identical on all 8 cores.  Everything
rank-dependent is carried by the *inputs* (own graph rows, own scales, own
FM slices fed pre-transposed by the host).  The pairwise AllGather output
is in true global order (rank r of each pair holds true half r), so
read-back is rank-independent too.
"""

import numpy as np
import ml_dtypes

B, P, Q, QN, D, H = 4, 2048, 1024, 32, 256, 128
NC_ = 8
PH, QH, QNH = P // 2, Q // 2, QN // 2      # own rows per core
PT, QT = PH // 128, QH // 128              # own p/q tiles (8, 4)
PJ, QJ = P // 128, Q // 128                # source p/q tiles (16, 8)

_BUILD_CACHE = {}


def _fix_multiwait(nc, mybir, limit=1):
    """walrus codegen allows only `limit` sync-waits on DMA-class (and some
    other) instructions.  Move extra waits onto same-engine NoOps inserted
    immediately before — the sequencer executes them in order, so semantics
    are unchanged."""
    ctr = 0
    for f in nc.m.functions:
        for bb in f.blocks:
            out = []
            changed = False
            for inst in bb.instructions:
                si = inst.sync_info
                if si is not None and len(si.on_wait) > limit:
                    waits = list(si.on_wait)
                    for w in waits[:-limit]:
                        ctr += 1
                        out.append(mybir.InstNoOp(
                            name=f"waitnop-{ctr}", ins=[], outs=[],
                            engine=inst.engine,
                            sync_info=mybir.SyncInfo(on_wait=[w], on_update=[]),
                        ))
                    inst.sync_info = mybir.SyncInfo(
                        on_wait=waits[-limit:], on_update=list(si.on_update))
                    changed = True
                out.append(inst)
            if changed:
                bb.instructions = out
    return ctr


def _build():
    import concourse.bass as bass
    import concourse.mybir as mybir
    import concourse.tile as tile

    bf16 = mybir.dt.bfloat16
    f32 = mybir.dt.float32
    i32 = mybir.dt.int32
    RELU = mybir.ActivationFunctionType.Relu
    SIGM = mybir.ActivationFunctionType.Sigmoid
    MUL = mybir.AluOpType.mult

    nc = bass.Bass(trn_type="TRN2", num_devices=NC_)

    # ---------------- DRAM I/O ----------------
    g_pp = nc.dram_tensor("g_pp", [PH, P], i32, kind="ExternalInput")
    g_qq = nc.dram_tensor("g_qq", [QH, Q], i32, kind="ExternalInput")
    p0_d = nc.dram_tensor("p0", [P, D], bf16, kind="ExternalInput")
    q0_d = nc.dram_tensor("q0", [Q, D], bf16, kind="ExternalInput")
    p0T_d = nc.dram_tensor("p0T", [D, P], bf16, kind="ExternalInput")
    q0T_d = nc.dram_tensor("q0T", [D, Q], bf16, kind="ExternalInput")
    p0oT_d = nc.dram_tensor("p0ownT", [D, PH], bf16, kind="ExternalInput")
    q0oT_d = nc.dram_tensor("q0ownT", [D, QH], bf16, kind="ExternalInput")
    qu0oT_d = nc.dram_tensor("qu0ownT", [D, QNH], bf16, kind="ExternalInput")
    wcat_p_d = nc.dram_tensor("wcat_p", [D, D + 1], bf16, kind="ExternalInput")
    wcat_q_d = nc.dram_tensor("wcat_q", [D, D + 1], bf16, kind="ExternalInput")
    wself_d = nc.dram_tensor("w_self", [D, D], bf16, kind="ExternalInput")
    wqd_d = nc.dram_tensor("w_qd", [D, D], bf16, kind="ExternalInput")
    wqup_d = nc.dram_tensor("w_qup", [D, D], bf16, kind="ExternalInput")
    wquq_d = nc.dram_tensor("w_quq", [D, D], bf16, kind="ExternalInput")
    wp_d = nc.dram_tensor("w_p", [D, H], bf16, kind="ExternalInput")
    wq_d = nc.dram_tensor("w_q", [D, H], bf16, kind="ExternalInput")
    wqu_d = nc.dram_tensor("w_qu", [D, H], bf16, kind="ExternalInput")
    bself_d = nc.dram_tensor("b_self_cols", [128, 2], f32, kind="ExternalInput")
    bp_d = nc.dram_tensor("bp_row", [1, H], bf16, kind="ExternalInput")
    bq_d = nc.dram_tensor("bq_row", [1, H], bf16, kind="ExternalInput")
    bqu_d = nc.dram_tensor("bqu_row", [1, H], bf16, kind="ExternalInput")
    bmask_p_d = nc.dram_tensor("bmask_p", [128, PJ], f32, kind="ExternalInput")
    bmask_q_d = nc.dram_tensor("bmask_q", [128, QJ], f32, kind="ExternalInput")
    sp_d = nc.dram_tensor("s_p_cols", [128, PT], f32, kind="ExternalInput")
    sq_d = nc.dram_tensor("s_q_cols", [128, QT], f32, kind="ExternalInput")
    sqrow_d = nc.dram_tensor("s_q_row", [1, QH], bf16, kind="ExternalInput")
    squrow_d = nc.dram_tensor("s_qu_row", [1, QNH], bf16, kind="ExternalInput")

    y_p = nc.dram_tensor("y_p", [PH, H], f32, kind="ExternalOutput")
    y_q = nc.dram_tensor("y_q", [QH, H], f32, kind="ExternalOutput")
    y_qu = nc.dram_tensor("y_qu", [QNH, H], f32, kind="ExternalOutput")

    with tile.TileContext(nc) as tc:
        with tc.tile_pool(name="persist", bufs=1) as pp_, \
             tc.tile_pool(name="stage", bufs=2) as stg, \
             tc.tile_pool(name="psx", bufs=4, space="PSUM") as psx, \
             tc.tile_pool(name="psm", bufs=4, space="PSUM") as psm, \
             tc.tile_pool(name="dram", bufs=1, space="DRAM") as dram:

            # ---------------- constant / state loads ----------------
            def load2(dr, shape, name):
                t = pp_.tile(shape, dr.dtype, tag=name)
                nc.sync.dma_start(t[:], dr[:].rearrange("(c p) n -> p c n", p=128))
                return t

            wcat_p = load2(wcat_p_d, [128, 2, D + 1], "wcat_p")
            wcat_q = load2(wcat_q_d, [128, 2, D + 1], "wcat_q")
            wself = load2(wself_d, [128, 2, D], "wself")
            wqd = load2(wqd_d, [128, 2, D], "wqd")
            wqup = load2(wqup_d, [128, 2, D], "wqup")
            wquq = load2(wquq_d, [128, 2, D], "wquq")
            wp = load2(wp_d, [128, 2, H], "wp")
            wq = load2(wq_d, [128, 2, H], "wq")
            wqu = load2(wqu_d, [128, 2, H], "wqu")
            p0_fm = load2(p0T_d, [128, 2, P], "p0_fm")
            q0_fm = load2(q0T_d, [128, 2, Q], "q0_fm")
            p0own_fm = load2(p0oT_d, [128, 2, PH], "p0own_fm")
            q0own_fm = load2(q0oT_d, [128, 2, QH], "q0own_fm")
            qu0own_fm = load2(qu0oT_d, [128, 2, QNH], "qu0own_fm")

            def load1(dr, shape, name):
                t = pp_.tile(shape, dr.dtype, tag=name)
                nc.sync.dma_start(t[:], dr[:])
                return t

            bself = load1(bself_d, [128, 2], "bself")
            bp_row = load1(bp_d, [1, H], "bp_row")
            bq_row = load1(bq_d, [1, H], "bq_row")
            bqu_row = load1(bqu_d, [1, H], "bqu_row")
            bmask_p = load1(bmask_p_d, [128, PJ], "bmask_p")
            bmask_q = load1(bmask_q_d, [128, QJ], "bmask_q")
            sp_cols = load1(sp_d, [128, PT], "sp_cols")
            sq_cols = load1(sq_d, [128, QT], "sq_cols")
            sq_row = load1(sqrow_d, [1, QH], "sq_row")
            squ_row = load1(squrow_d, [1, QNH], "squ_row")

            ones_row = pp_.tile([1, 128], bf16, tag="ones_row")
            nc.vector.memset(ones_row[:], 1.0)

            p0_tm = pp_.tile([128, PJ, D], bf16, tag="p0_tm")
            nc.sync.dma_start(p0_tm[:], p0_d[:].rearrange("(t p) d -> p t d", p=128))
            q0_tm = pp_.tile([128, QJ, D], bf16, tag="q0_tm")
            nc.sync.dma_start(q0_tm[:], q0_d[:].rearrange("(t p) d -> p t d", p=128))

            # step-1 outputs (own rows, FM), step-2 state
            p1own_fm = pp_.tile([128, 2, PH], bf16, tag="p1own_fm")
            q1own_fm = pp_.tile([128, 2, QH], bf16, tag="q1own_fm")
            qu1own_fm = pp_.tile([128, 2, QNH], bf16, tag="qu1own_fm")
            p1_fm = pp_.tile([128, 2, P], bf16, tag="p1_fm")
            q1_fm = pp_.tile([128, 2, Q], bf16, tag="q1_fm")
            # ptm_x[jp, r, c, jt, dp] = p1[r*PH + jt*128 + jp, c*128 + dp]
            ptm_x = pp_.tile([128, 2, 2, PJ // 2, 128], bf16, tag="ptm_x")
            qtm_x = pp_.tile([128, 2, 2, QJ // 2, 128], bf16, tag="qtm_x")
            p2own_fm = pp_.tile([128, 2, PH], bf16, tag="p2own_fm")
            q2own_fm = pp_.tile([128, 2, QH], bf16, tag="q2own_fm")
            qu2own_fm = pp_.tile([128, 2, QNH], bf16, tag="qu2own_fm")

            # ---------------- graph load / scale / transpose ----------------
            # GT_pp[jp, t, jt, i'] = s_p[t*128+i'] * g_pp[t*128+i', jt*128+jp]
            gt_pp = pp_.tile([128, PT, PJ, 128], bf16, tag="gt_pp")
            gt_qq = pp_.tile([128, QT, QJ, 128], bf16, tag="gt_qq")

            for t in range(PT):
                gi = stg.tile([128, P], i32, tag="g_int")
                nc.sync.dma_start(gi[:], g_pp[t * 128:(t + 1) * 128, :])
                gb = stg.tile([128, P], bf16, tag="g_bf")
                nc.vector.tensor_scalar(gb[:], gi[:], sp_cols[:, t:t + 1], None, MUL)
                nc.sync.dma_start(gt_pp[:, t], gb[:], transpose=True)
            for t in range(QT):
                gi = stg.tile([128, Q], i32, tag="g_int_q")
                nc.sync.dma_start(gi[:], g_qq[t * 128:(t + 1) * 128, :])
                gb = stg.tile([128, Q], bf16, tag="g_bf_q")
                nc.vector.tensor_scalar(gb[:], gi[:], sq_cols[:, t:t + 1], None, MUL)
                nc.sync.dma_start(gt_qq[:, t], gb[:], transpose=True)

            # all-gather bounce buffers (bf16): blocks [c, 128, PH | QH]
            cin = dram.tile([2, 128, PH + QH], bf16)
            cout = dram.tile([4, 128, PH + QH], bf16)

            def step(k, pfm, qfm, pown, qown, quown, ptm, qtm,
                     xpp, xqq, out_p, out_q, out_qu):
                # -- transforms X~ + z (rhs = [W | W_node]), per source tile --
                wbf_p = pp_.tile([128, PJ], bf16, tag=f"wbf_p{k}")
                wbf_q = pp_.tile([128, QJ], bf16, tag=f"wbf_q{k}")
                for t in range(PJ):
                    px = psx.tile([128, D + 1], f32, tag="psx")
                    for c in range(2):
                        nc.tensor.matmul(px[:], pfm[:, c, t * 128:(t + 1) * 128],
                                         wcat_p[:, c, :], start=(c == 0), stop=(c == 1))
                    wcol = stg.tile([128, 1], f32, tag="wcol")
                    nc.scalar.activation(wcol[:], px[:, D:D + 1], SIGM,
                                         bias=bmask_p[:, t:t + 1], scale=1.0)
                    nc.vector.tensor_copy(wbf_p[:, t:t + 1], wcol[:])
                    nc.vector.tensor_scalar(xpp[:, t], px[:, 0:D], wcol[:], None, MUL)
                for t in range(QJ):
                    px = psx.tile([128, D + 1], f32, tag="psx")
                    for c in range(2):
                        nc.tensor.matmul(px[:], qfm[:, c, t * 128:(t + 1) * 128],
                                         wcat_q[:, c, :], start=(c == 0), stop=(c == 1))
                    wcol = stg.tile([128, 1], f32, tag="wcol")
                    nc.scalar.activation(wcol[:], px[:, D:D + 1], SIGM,
                                         bias=bmask_q[:, t:t + 1], scale=1.0)
                    nc.vector.tensor_copy(wbf_q[:, t:t + 1], wcol[:])
                    nc.vector.tensor_scalar(xqq[:, t], px[:, 0:D], wcol[:], None, MUL)

                # -- u reductions: u[d] = sum_j w_j node[j, d] --
                up_ps = psx.tile([128, 2], f32, tag="psx")
                for c in range(2):
                    for t in range(PJ):
                        nc.tensor.matmul(up_ps[:, c:c + 1], ptm(t, c),
                                         wbf_p[:, t:t + 1],
                                         start=(t == 0), stop=(t == PJ - 1))
                uq_ps = psx.tile([128, 2], f32, tag="psx")
                for c in range(2):
                    for t in range(QJ):
                        nc.tensor.matmul(uq_ps[:, c:c + 1], qtm(t, c),
                                         wbf_q[:, t:t + 1],
                                         start=(t == 0), stop=(t == QJ - 1))
                up_bf = stg.tile([128, 2], bf16, tag="up_bf")
                uq_bf = stg.tile([128, 2], bf16, tag="uq_bf")
                nc.vector.tensor_copy(up_bf[:], up_ps[:])
                nc.vector.tensor_copy(uq_bf[:], uq_ps[:])
                # v matvecs: v_qd = u_p @ W_qd ; v_upq = u_p @ W_qup + u_q @ W_quq
                vqd_ps = psx.tile([1, D], f32, tag="psx")
                for c in range(2):
                    nc.tensor.matmul(vqd_ps[:], up_bf[:, c:c + 1], wqd[:, c, :],
                                     start=(c == 0), stop=(c == 1))
                vupq_ps = psx.tile([1, D], f32, tag="psx")
                for c in range(2):
                    nc.tensor.matmul(vupq_ps[:], up_bf[:, c:c + 1], wqup[:, c, :],
                                     start=(c == 0), stop=False)
                for c in range(2):
                    nc.tensor.matmul(vupq_ps[:], uq_bf[:, c:c + 1], wquq[:, c, :],
                                     start=False, stop=(c == 1))
                vqd_bf = stg.tile([1, D], bf16, tag="vqd")
                vupq_bf = stg.tile([1, D], bf16, tag="vupq")
                nc.vector.tensor_copy(vqd_bf[:], vqd_ps[:])
                nc.vector.tensor_copy(vupq_bf[:], vupq_ps[:])

                # -- main update (own rows, FM) --
                for c in range(2):
                    for ic in range(2):  # two 512-chunks of own p rows
                        ps = psm.tile([128, 512], f32, tag="psm")
                        for cc in range(2):
                            nc.tensor.matmul(
                                ps[:], wself[:, cc, c * 128:(c + 1) * 128],
                                pown[:, cc, ic * 512:ic * 512 + 512],
                                start=(cc == 0), stop=False)
                        for jt in range(PJ):
                            nc.tensor.matmul(
                                ps[:], xpp[:, jt, c * 128:(c + 1) * 128],
                                gt_pp[:, ic * 4:(ic + 1) * 4, jt, :],
                                start=False, stop=(jt == PJ - 1))
                        nc.scalar.activation(
                            out_p[:, c, ic * 512:ic * 512 + 512], ps[:], RELU,
                            bias=bself[:, c:c + 1], scale=1.0)
                    # q rows: one 512 chunk
                    ps = psm.tile([128, 512], f32, tag="psm")
                    for cc in range(2):
                        nc.tensor.matmul(
                            ps[:], wself[:, cc, c * 128:(c + 1) * 128],
                            qown[:, cc, :], start=(cc == 0), stop=False)
                    nc.tensor.matmul(ps[:], vqd_bf[:, c * 128:(c + 1) * 128],
                                     sq_row[:], start=False, stop=False)
                    for jt in range(QJ):
                        nc.tensor.matmul(
                            ps[:], xqq[:, jt, c * 128:(c + 1) * 128],
                            gt_qq[:, :, jt, :],
                            start=False, stop=(jt == QJ - 1))
                    nc.scalar.activation(out_q[:, c, :], ps[:], RELU,
                                         bias=bself[:, c:c + 1], scale=1.0)
                    # qu rows (own 16)
                    ps = psx.tile([128, QNH], f32, tag="psx")
                    for cc in range(2):
                        nc.tensor.matmul(
                            ps[:], wself[:, cc, c * 128:(c + 1) * 128],
                            quown[:, cc, :], start=(cc == 0), stop=False)
                    nc.tensor.matmul(ps[:], vupq_bf[:, c * 128:(c + 1) * 128],
                                     squ_row[:], start=False, stop=True)
                    nc.scalar.activation(out_qu[:, c, :], ps[:], RELU,
                                         bias=bself[:, c:c + 1], scale=1.0)

            # ---------------- step 1 ----------------
            xpp1 = pp_.tile([128, PJ, D], bf16, tag="xpp")
            xqq1 = pp_.tile([128, QJ, D], bf16, tag="xqq")
            step(1, p0_fm, q0_fm, p0own_fm, q0own_fm, qu0own_fm,
                 lambda t, c: p0_tm[:, t, c * 128:(c + 1) * 128],
                 lambda t, c: q0_tm[:, t, c * 128:(c + 1) * 128],
                 xpp1, xqq1, p1own_fm, q1own_fm, qu1own_fm)

            # ---------------- exchange (pairwise all-gather) ----------------
            for c in range(2):
                nc.sync.dma_start(cin[c, :, 0:PH], p1own_fm[:, c, :])
                nc.sync.dma_start(cin[c, :, PH:PH + QH], q1own_fm[:, c, :])
            nc.gpsimd.collective_compute(
                "AllGather", mybir.AluOpType.bypass,
                replica_groups=[[0, 1], [2, 3], [4, 5], [6, 7]],
                ins=[cin.opt()], outs=[cout.opt()])
            # read back in true global order (rank r of the pair = half r)
            for r in range(2):
                for c in range(2):
                    nc.sync.dma_start(p1_fm[:, c, r * PH:(r + 1) * PH],
                                      cout[r * 2 + c, :, 0:PH])
                    nc.sync.dma_start(q1_fm[:, c, r * QH:(r + 1) * QH],
                                      cout[r * 2 + c, :, PH:PH + QH])
                    nc.sync.dma_start(ptm_x[:, r, c], cout[r * 2 + c, :, 0:PH],
                                      transpose=True)
                    nc.sync.dma_start(qtm_x[:, r, c], cout[r * 2 + c, :, PH:PH + QH],
                                      transpose=True)

            # ---------------- step 2 ----------------
            xpp2 = pp_.tile([128, PJ, D], bf16, tag="xpp")
            xqq2 = pp_.tile([128, QJ, D], bf16, tag="xqq")
            step(2, p1_fm, q1_fm, p1own_fm, q1own_fm, qu1own_fm,
                 lambda t, c: ptm_x[:, t // (PJ // 2), c, t % (PJ // 2), :],
                 lambda t, c: qtm_x[:, t // (QJ // 2), c, t % (QJ // 2), :],
                 xpp2, xqq2, p2own_fm, q2own_fm, qu2own_fm)

            # ---------------- final projections ----------------
            def project(fm, w2, brow, ydram, ntile):
                for t in range(ntile):
                    ps = psx.tile([128, H], f32, tag="psx")
                    for c in range(2):
                        nc.tensor.matmul(ps[:], fm[:, c, t * 128:(t + 1) * 128],
                                         w2[:, c, :], start=(c == 0), stop=False)
                    nc.tensor.matmul(ps[:], ones_row[:], brow[:],
                                     start=False, stop=True)
                    ot = stg.tile([128, H], f32, tag="proj_out")
                    nc.scalar.copy(ot[:], ps[:])
                    nc.sync.dma_start(ydram[t * 128:(t + 1) * 128, :], ot[:])

            project(p2own_fm, wp, bp_row, y_p, PT)
            project(q2own_fm, wq, bq_row, y_q, QT)
            ps = psx.tile([QNH, H], f32, tag="psx")
            for c in range(2):
                nc.tensor.matmul(ps[:], qu2own_fm[:, c, :], wqu[:, c, :],
                                 start=(c == 0), stop=False)
            nc.tensor.matmul(ps[:], ones_row[:, 0:QNH], bqu_row[:],
                             start=False, stop=True)
            ot = stg.tile([QNH, H], f32, tag="proj_out_qu")
            nc.scalar.copy(ot[:], ps[:])
            nc.sync.dma_start(y_qu[:], ot[:])

    _fix_multiwait(nc, mybir)
    return nc


def _host_prep(inputs):
    f32 = np.float32
    bf = ml_dtypes.bfloat16
    pm = inputs["p_node_mask"].astype(f32)
    qm = inputs["q_node_mask"].astype(f32)
    qum = inputs["question_node_mask"].astype(f32)
    gpp = inputs["pp_graph"]
    gqq = inputs["qq_graph"]

    nb_p = pm * np.einsum("bij,bj->bi", gpp.astype(f32), pm)
    nb_p = np.where(nb_p >= 1, nb_p, 1.0)
    npm = pm.sum(-1)
    nqm = qm.sum(-1)
    nb_q = qm * (npm[:, None] + np.einsum("bij,bj->bi", gqq.astype(f32), qm))
    nb_q = np.where(nb_q >= 1, nb_q, 1.0)
    nb_qu = qum * (npm[:, None] + nqm[:, None])
    nb_qu = np.where(nb_qu >= 1, nb_qu, 1.0)
    s_p = pm / nb_p
    s_q = qm / nb_q
    s_qu = qum / nb_qu

    W_node = inputs["W_node"].astype(f32)
    b_node = float(np.asarray(inputs["b_node"]).reshape(-1)[0])
    wcat_p = np.concatenate([inputs["W_pp"], W_node], axis=1).astype(bf)
    wcat_q = np.concatenate([inputs["W_qq"], W_node], axis=1).astype(bf)
    b_self_cols = np.ascontiguousarray(
        inputs["b_self"].astype(f32).reshape(2, 128).T)

    def cols(v):  # [N] -> [128, N//128] with v[t*128+p] at [p, t]
        return np.ascontiguousarray(v.reshape(-1, 128).T)

    def ct(a, dt=bf):  # contiguous cast
        return np.ascontiguousarray(a).astype(dt)

    in_maps = []
    for k in range(NC_):
        b, h = divmod(k, 2)
        sp_ = slice(h * PH, (h + 1) * PH)
        sq_ = slice(h * QH, (h + 1) * QH)
        squ = slice(h * QNH, (h + 1) * QNH)
        p0b = inputs["p_node"][b].astype(bf)
        q0b = inputs["q_node"][b].astype(bf)
        m = dict(
            g_pp=np.ascontiguousarray(gpp[b, sp_]),
            g_qq=np.ascontiguousarray(gqq[b, sq_]),
            p0=p0b, q0=q0b,
            p0T=np.ascontiguousarray(p0b.T),
            q0T=np.ascontiguousarray(q0b.T),
            p0ownT=np.ascontiguousarray(p0b[sp_].T),
            q0ownT=np.ascontiguousarray(q0b[sq_].T),
            qu0ownT=ct(inputs["question_node"][b][squ].T),
            wcat_p=wcat_p, wcat_q=wcat_q,
            w_self=ct(inputs["W_self"]),
            w_qd=ct(inputs["W_qd"]),
            w_qup=ct(inputs["W_qup"]),
            w_quq=ct(inputs["W_quq"]),
            w_p=ct(inputs["W_p"]),
            w_q=ct(inputs["W_q"]),
            w_qu=ct(inputs["W_qu"]),
            b_self_cols=b_self_cols,
            bp_row=inputs["b_p"].astype(bf).reshape(1, H),
            bq_row=inputs["b_q"].astype(bf).reshape(1, H),
            bqu_row=inputs["b_qu"].astype(bf).reshape(1, H),
            bmask_p=cols(b_node + (pm[b] - 1.0) * 30.0),
            bmask_q=cols(b_node + (qm[b] - 1.0) * 30.0),
            s_p_cols=cols(s_p[b, sp_]),
            s_q_cols=cols(s_q[b, sq_]),
            s_q_row=s_q[b, sq_].astype(bf).reshape(1, QH),
            s_qu_row=s_qu[b, squ].astype(bf).reshape(1, QNH),
        )
        in_maps.append(m)
    return in_maps


def kernel(**inputs):
    from concourse.bass_utils import run_bass_kernel_spmd

    if "nc" not in _BUILD_CACHE:
        _BUILD_CACHE["nc"] = _build()
    nc = _BUILD_CACHE["nc"]
    in_maps = _host_prep(inputs)
    res = run_bass_kernel_spmd(nc, in_maps, core_ids=list(range(NC_)))
    r = res.results

    p_out = np.zeros((B, P, H), np.float32)
    q_out = np.zeros((B, Q, H), np.float32)
    qu_out = np.zeros((B, QN, H), np.float32)
    for k in range(NC_):
        b, h = divmod(k, 2)
        p_out[b, h * PH:(h + 1) * PH] = r[k]["y_p"]
        q_out[b, h * QH:(h + 1) * QH] = r[k]["y_q"]
        qu_out[b, h * QNH:(h + 1) * QNH] = r[k]["y_qu"]
    return p_out, q_out, qu_out


# revision 7
# speedup vs baseline: 1.1630x; 1.1630x over previous
"""Trainium2 Bass kernel for nn_CharmGNN (2-step GNN message passing).

Sharding: 8 cores = 4 batches x 2 row-halves. Each core owns half the
p-rows (1024), q-rows (512), qu-rows (16) of one batch element.

Key algebra:
  - qp / qu_p / qu_q "adjacencies" are rank-1 (outer products of masks):
    their matmuls collapse to weighted reductions u[d] = sum_j w_j node[j,d]
    followed by tiny matvecs v = u @ W.
  - per-row scale s_i = mask_i / nb_i is folded into the graph during the
    int32->bf16 conversion (graph fed column-sliced, so dest rows are on
    the free axis and s becomes a materialized row-constant multiplier).
  - source weights w_j = pm_j * sigmoid(z_j + b_node) are computed as
    sigmoid(z_j + b_node + (pm_j - 1) * 30)   (sigmoid(-30) ~ 1e-13 ~ 0).
  - all updates run feature-major (FM: [D on partitions, tokens free]);
    token-major (TM) copies are only needed by the u-reductions and come
    from a DMA-transpose of the all-gathered DRAM buffer.

SPMD cleanliness: the NEFF is identical on all 8 cores. Everything
rank-dependent is carried by the inputs (own graph columns, own scales,
own FM slices fed pre-transposed by the host). The pairwise AllGather
output is in true global order (rank r of each pair holds true half r),
so read-back is rank-independent.

DMA discipline: walrus allows 1 sync-wait per DMA instruction
(_fix_multiwait moves extras onto NoOps), HWDGE rings are per-engine FIFO
and each DMA carries ~2us fixed cost -> few, large DMAs, alternated
between the SP (nc.sync) and ACT (nc.scalar) rings.
"""

import numpy as np
import ml_dtypes

B, P, Q, QN, D, H = 4, 2048, 1024, 32, 256, 128
NC_ = 8
PH, QH, QNH = P // 2, Q // 2, QN // 2      # own rows per core
PT, QT = PH // 128, QH // 128              # own p/q tiles (8, 4)
PJ, QJ = P // 128, Q // 128                # source p/q tiles (16, 8)
SW = PH + QH                               # gathered state width per (r, c)

# -------- packed input blob layouts (host and device must agree) --------
_BF_SLICES = {}
_off = 0
for _nm, _w in [("wcat_p", 2 * (D + 1)), ("wcat_q", 2 * (D + 1)),
                ("wself", 2 * D), ("wqd", 2 * D), ("wqup", 2 * D),
                ("wquq", 2 * D), ("wp", 2 * H), ("wq", 2 * H), ("wqu", 2 * H),
                ("rows", H * 3 + QH + QNH + 128)]:
    _BF_SLICES[_nm] = (_off, _w)
    _off += _w
WBF = _off
_ROWS = {}
_off = 0
for _nm, _w in [("bp", H), ("bq", H), ("bqu", H), ("sqrow", QH),
                ("squrow", QNH), ("ones", 128)]:
    _ROWS[_nm] = (_off, _w)
    _off += _w
_F32_SLICES = {}
_off = 0
for _nm, _w in [("bself", 2), ("bmask_p", PJ), ("bmask_q", QJ),
                ("smp", PH), ("smq", QH)]:
    _F32_SLICES[_nm] = (_off, _w)
    _off += _w
WF32 = _off
_FM_SLICES = {}
_off = 0
for _nm, _w in [("p0", P), ("q0", Q), ("p0own", PH), ("q0own", QH),
                ("qu0own", QNH)]:
    _FM_SLICES[_nm] = (_off, _w)
    _off += _w
NFM = _off

_BUILD_CACHE = {}


def _fix_multiwait(nc, mybir, limit=1):
    """walrus codegen allows only `limit` sync-waits on DMA-class
    instructions.  Move extra waits onto same-engine NoOps inserted
    immediately before — the sequencer executes them in order, so semantics
    are unchanged."""
    ctr = 0
    for f in nc.m.functions:
        for bb in f.blocks:
            out = []
            changed = False
            for inst in bb.instructions:
                si = inst.sync_info
                if si is not None and len(si.on_wait) > limit:
                    waits = list(si.on_wait)
                    for w in waits[:-limit]:
                        ctr += 1
                        out.append(mybir.InstNoOp(
                            name=f"waitnop-{ctr}", ins=[], outs=[],
                            engine=inst.engine,
                            sync_info=mybir.SyncInfo(on_wait=[w], on_update=[]),
                        ))
                    inst.sync_info = mybir.SyncInfo(
                        on_wait=waits[-limit:], on_update=list(si.on_update))
                    changed = True
                out.append(inst)
            if changed:
                bb.instructions = out
    return ctr


def _build():
    import concourse.bass as bass
    import concourse.mybir as mybir
    import concourse.tile as tile

    bf16 = mybir.dt.bfloat16
    f32 = mybir.dt.float32
    i32 = mybir.dt.int32
    RELU = mybir.ActivationFunctionType.Relu
    SIGM = mybir.ActivationFunctionType.Sigmoid
    MUL = mybir.AluOpType.mult

    nc = bass.Bass(trn_type="TRN2", num_devices=NC_)

    g_ppT = nc.dram_tensor("g_ppT", [P, PH], i32, kind="ExternalInput")
    g_qqT = nc.dram_tensor("g_qqT", [Q, QH], i32, kind="ExternalInput")
    wbf_d = nc.dram_tensor("wblob_bf", [128, WBF], bf16, kind="ExternalInput")
    wf32_d = nc.dram_tensor("wblob_f32", [128, WF32], f32, kind="ExternalInput")
    nfm_d = nc.dram_tensor("node_fm", [128, 2, NFM], bf16, kind="ExternalInput")
    ntm_d = nc.dram_tensor("node_tm", [128, PJ + QJ, D], bf16,
                           kind="ExternalInput")

    y_p = nc.dram_tensor("y_p", [PH, H], f32, kind="ExternalOutput")
    y_q = nc.dram_tensor("y_q", [QH, H], f32, kind="ExternalOutput")
    y_qu = nc.dram_tensor("y_qu", [QNH, H], f32, kind="ExternalOutput")

    with tile.TileContext(nc) as tc:
        with tc.tile_pool(name="persist", bufs=1) as pp_, \
             tc.tile_pool(name="stage", bufs=2) as stg, \
             tc.tile_pool(name="psx", bufs=4, space="PSUM") as psx, \
             tc.tile_pool(name="psm", bufs=4, space="PSUM") as psm, \
             tc.tile_pool(name="dram", bufs=1, space="DRAM") as dram:

            # ---------------- blob loads (4 DMAs) ----------------
            wbf = pp_.tile([128, WBF], bf16, tag="wbf")
            nc.sync.dma_start(wbf[:], wbf_d[:])
            wf32 = pp_.tile([128, WF32], f32, tag="wf32")
            nc.scalar.dma_start(wf32[:], wf32_d[:])
            nfm = pp_.tile([128, 2, NFM], bf16, tag="nfm")
            nc.sync.dma_start(nfm[:], nfm_d[:])
            ntm = pp_.tile([128, PJ + QJ, D], bf16, tag="ntm")
            nc.scalar.dma_start(ntm[:], ntm_d[:])

            def bfv(nm):
                off, w = _BF_SLICES[nm]
                return wbf[:, off:off + w].rearrange("p (c n) -> p c n", c=2)

            wcat_p, wcat_q = bfv("wcat_p"), bfv("wcat_q")
            wself = bfv("wself")
            wqd, wqup, wquq = bfv("wqd"), bfv("wqup"), bfv("wquq")
            wp, wq, wqu = bfv("wp"), bfv("wq"), bfv("wqu")
            rows_off = _BF_SLICES["rows"][0]

            def rowv(nm):
                off, w = _ROWS[nm]
                return wbf[0:1, rows_off + off:rows_off + off + w]

            bp_row, bq_row, bqu_row = rowv("bp"), rowv("bq"), rowv("bqu")
            sq_row, squ_row, ones_row = rowv("sqrow"), rowv("squrow"), rowv("ones")

            def fv(nm):
                off, w = _F32_SLICES[nm]
                return wf32[:, off:off + w]

            bself, bmask_p, bmask_q = fv("bself"), fv("bmask_p"), fv("bmask_q")
            s_mat_p, s_mat_q = fv("smp"), fv("smq")

            def fmv(nm):
                off, w = _FM_SLICES[nm]
                return nfm[:, :, off:off + w]

            p0_fm, q0_fm = fmv("p0"), fmv("q0")
            p0own_fm, q0own_fm, qu0own_fm = fmv("p0own"), fmv("q0own"), fmv("qu0own")
            p0_tm = ntm[:, 0:PJ, :]
            q0_tm = ntm[:, PJ:PJ + QJ, :]

            # ------------- graph load / scale (already transposed) -------------
            # gt_pp[jp, jt, i] = s_p[i] * g[i, jt*128 + jp]   (i = own row)
            gt_pp = pp_.tile([128, PJ, PH], bf16, tag="gt_pp")
            gt_qq = pp_.tile([128, QJ, QH], bf16, tag="gt_qq")
            GCH = 4  # jt-tiles per graph DMA chunk
            for ck in range(PJ // GCH):
                gi = stg.tile([128, GCH, PH], i32, tag="g_int")
                eng = nc.sync if ck % 2 == 0 else nc.scalar
                eng.dma_start(gi[:], g_ppT[ck * GCH * 128:(ck + 1) * GCH * 128, :]
                              .rearrange("(t p) n -> p t n", p=128))
                nc.vector.tensor_tensor(
                    gt_pp[:, ck * GCH:(ck + 1) * GCH, :], gi[:],
                    s_mat_p[:, None, :].to_broadcast([128, GCH, PH]), MUL)
            for ck in range(QJ // GCH):
                gi = stg.tile([128, GCH, QH], i32, tag="g_int_q")
                eng = nc.sync if ck % 2 == 0 else nc.scalar
                eng.dma_start(gi[:], g_qqT[ck * GCH * 128:(ck + 1) * GCH * 128, :]
                              .rearrange("(t p) n -> p t n", p=128))
                nc.vector.tensor_tensor(
                    gt_qq[:, ck * GCH:(ck + 1) * GCH, :], gi[:],
                    s_mat_q[:, None, :].to_broadcast([128, GCH, QH]), MUL)

            # ---------------- state tensors ----------------
            own1 = pp_.tile([128, 2, SW], bf16, tag="own1")
            qu1own_fm = pp_.tile([128, 2, QNH], bf16, tag="qu1own_fm")
            st1 = pp_.tile([128, 2, 2, SW], bf16, tag="st1")
            # tm_x[jp, r, c, jt, dp]: jt<8 -> p1 tokens, jt>=8 -> q1 tokens
            tm_x = pp_.tile([128, 2, 2, 12, 128], bf16, tag="tm_x")
            p2own_fm = pp_.tile([128, 2, PH], bf16, tag="p2own_fm")
            q2own_fm = pp_.tile([128, 2, QH], bf16, tag="q2own_fm")
            qu2own_fm = pp_.tile([128, 2, QNH], bf16, tag="qu2own_fm")

            cin = dram.tile([2, 128, SW], bf16)
            cout = dram.tile([4, 128, SW], bf16)

            def step(k, pfm, qfm, pown, qown, quown, ptm, qtm,
                     xpp, xqq, out_p, out_q, out_qu):
                wbf_p = pp_.tile([128, PJ], bf16, tag=f"wbf_p{k}")
                wbf_q = pp_.tile([128, QJ], bf16, tag=f"wbf_q{k}")
                # -- transforms X~ + z (rhs = [W | W_node]) --
                for t in range(PJ):
                    px = psx.tile([128, D + 1], f32, tag="psx")
                    for c in range(2):
                        nc.tensor.matmul(px[:], pfm(t, c), wcat_p[:, c, :],
                                         start=(c == 0), stop=(c == 1))
                    wcol = stg.tile([128, 1], f32, tag="wcol")
                    nc.scalar.activation(wcol[:], px[:, D:D + 1], SIGM,
                                         bias=bmask_p[:, t:t + 1], scale=1.0)
                    nc.vector.tensor_copy(wbf_p[:, t:t + 1], wcol[:])
                    nc.vector.tensor_scalar(xpp[:, t], px[:, 0:D], wcol[:], None, MUL)
                for t in range(QJ):
                    px = psx.tile([128, D + 1], f32, tag="psx")
                    for c in range(2):
                        nc.tensor.matmul(px[:], qfm(t, c), wcat_q[:, c, :],
                                         start=(c == 0), stop=(c == 1))
                    wcol = stg.tile([128, 1], f32, tag="wcol")
                    nc.scalar.activation(wcol[:], px[:, D:D + 1], SIGM,
                                         bias=bmask_q[:, t:t + 1], scale=1.0)
                    nc.vector.tensor_copy(wbf_q[:, t:t + 1], wcol[:])
                    nc.vector.tensor_scalar(xqq[:, t], px[:, 0:D], wcol[:], None, MUL)

                # -- u reductions --
                up_ps = psx.tile([128, 2], f32, tag="psx")
                for c in range(2):
                    for t in range(PJ):
                        nc.tensor.matmul(up_ps[:, c:c + 1], ptm(t, c),
                                         wbf_p[:, t:t + 1],
                                         start=(t == 0), stop=(t == PJ - 1))
                uq_ps = psx.tile([128, 2], f32, tag="psx")
                for c in range(2):
                    for t in range(QJ):
                        nc.tensor.matmul(uq_ps[:, c:c + 1], qtm(t, c),
                                         wbf_q[:, t:t + 1],
                                         start=(t == 0), stop=(t == QJ - 1))
                up_bf = stg.tile([128, 2], bf16, tag="up_bf")
                uq_bf = stg.tile([128, 2], bf16, tag="uq_bf")
                nc.vector.tensor_copy(up_bf[:], up_ps[:])
                nc.vector.tensor_copy(uq_bf[:], uq_ps[:])
                vqd_ps = psx.tile([1, D], f32, tag="psx")
                for c in range(2):
                    nc.tensor.matmul(vqd_ps[:], up_bf[:, c:c + 1], wqd[:, c, :],
                                     start=(c == 0), stop=(c == 1))
                vupq_ps = psx.tile([1, D], f32, tag="psx")
                for c in range(2):
                    nc.tensor.matmul(vupq_ps[:], up_bf[:, c:c + 1], wqup[:, c, :],
                                     start=(c == 0), stop=False)
                for c in range(2):
                    nc.tensor.matmul(vupq_ps[:], uq_bf[:, c:c + 1], wquq[:, c, :],
                                     start=False, stop=(c == 1))
                vqd_bf = stg.tile([1, D], bf16, tag="vqd")
                vupq_bf = stg.tile([1, D], bf16, tag="vupq")
                nc.vector.tensor_copy(vqd_bf[:], vqd_ps[:])
                nc.vector.tensor_copy(vupq_bf[:], vupq_ps[:])

                # -- main update (own rows, FM) --
                for c in range(2):
                    for ic in range(2):
                        ps = psm.tile([128, 512], f32, tag="psm")
                        for cc in range(2):
                            nc.tensor.matmul(
                                ps[:], wself[:, cc, c * 128:(c + 1) * 128],
                                pown[:, cc, ic * 512:ic * 512 + 512],
                                start=(cc == 0), stop=False)
                        for jt in range(PJ):
                            nc.tensor.matmul(
                                ps[:], xpp[:, jt, c * 128:(c + 1) * 128],
                                gt_pp[:, jt, ic * 512:ic * 512 + 512],
                                start=False, stop=(jt == PJ - 1))
                        nc.scalar.activation(
                            out_p(c, ic * 512, 512), ps[:], RELU,
                            bias=bself[:, c:c + 1], scale=1.0)
                    ps = psm.tile([128, 512], f32, tag="psm")
                    for cc in range(2):
                        nc.tensor.matmul(
                            ps[:], wself[:, cc, c * 128:(c + 1) * 128],
                            qown[:, cc, :], start=(cc == 0), stop=False)
                    nc.tensor.matmul(ps[:], vqd_bf[:, c * 128:(c + 1) * 128],
                                     sq_row[:], start=False, stop=False)
                    for jt in range(QJ):
                        nc.tensor.matmul(
                            ps[:], xqq[:, jt, c * 128:(c + 1) * 128],
                            gt_qq[:, jt, :],
                            start=False, stop=(jt == QJ - 1))
                    nc.scalar.activation(out_q(c, 0, QH), ps[:], RELU,
                                         bias=bself[:, c:c + 1], scale=1.0)
                    ps = psx.tile([128, QNH], f32, tag="psx")
                    for cc in range(2):
                        nc.tensor.matmul(
                            ps[:], wself[:, cc, c * 128:(c + 1) * 128],
                            quown[:, cc, :], start=(cc == 0), stop=False)
                    nc.tensor.matmul(ps[:], vupq_bf[:, c * 128:(c + 1) * 128],
                                     squ_row[:], start=False, stop=True)
                    nc.scalar.activation(out_qu(c), ps[:], RELU,
                                         bias=bself[:, c:c + 1], scale=1.0)

            # ---------------- step 1 ----------------
            xpp1 = pp_.tile([128, PJ, D], bf16, tag="xpp")
            xqq1 = pp_.tile([128, QJ, D], bf16, tag="xqq")
            step(1,
                 lambda t, c: p0_fm[:, c, t * 128:(t + 1) * 128],
                 lambda t, c: q0_fm[:, c, t * 128:(t + 1) * 128],
                 p0own_fm, q0own_fm, qu0own_fm,
                 lambda t, c: p0_tm[:, t, c * 128:(c + 1) * 128],
                 lambda t, c: q0_tm[:, t, c * 128:(c + 1) * 128],
                 xpp1, xqq1,
                 lambda c, off, ln: own1[:, c, off:off + ln],
                 lambda c, off, ln: own1[:, c, PH + off:PH + off + ln],
                 lambda c: qu1own_fm[:, c, :])

            # ---------------- exchange (pairwise all-gather) ----------------
            nc.sync.dma_start(cin[:].rearrange("c p n -> p c n"), own1[:])
            nc.gpsimd.collective_compute(
                "AllGather", mybir.AluOpType.bypass,
                replica_groups=[[0, 1], [2, 3], [4, 5], [6, 7]],
                ins=[cin.opt()], outs=[cout.opt()])
            for r in range(2):
                for c in range(2):
                    e1 = nc.sync if c == 0 else nc.scalar
                    e2 = nc.scalar if c == 0 else nc.sync
                    e1.dma_start(st1[:, c, r, :], cout[r * 2 + c, :, :])
                    e2.dma_start(tm_x[:, r, c], cout[r * 2 + c, :, :],
                                 transpose=True)

            # ---------------- step 2 ----------------
            xpp2 = pp_.tile([128, PJ, D], bf16, tag="xpp")
            xqq2 = pp_.tile([128, QJ, D], bf16, tag="xqq")
            step(2,
                 lambda t, c: st1[:, c, t // PT, (t % PT) * 128:(t % PT + 1) * 128],
                 lambda t, c: st1[:, c, t // QT,
                                  PH + (t % QT) * 128:PH + (t % QT + 1) * 128],
                 own1[:, :, 0:PH], own1[:, :, PH:SW], qu1own_fm,
                 lambda t, c: tm_x[:, t // PT, c, t % PT, :],
                 lambda t, c: tm_x[:, t // QT, c, 8 + t % QT, :],
                 xpp2, xqq2,
                 lambda c, off, ln: p2own_fm[:, c, off:off + ln],
                 lambda c, off, ln: q2own_fm[:, c, off:off + ln],
                 lambda c: qu2own_fm[:, c, :])

            # ---------------- final projections ----------------
            def project(fm, w2, brow, ysb, ntile):
                for t in range(ntile):
                    ps = psx.tile([128, H], f32, tag="psx")
                    for c in range(2):
                        nc.tensor.matmul(ps[:], fm[:, c, t * 128:(t + 1) * 128],
                                         w2[:, c, :], start=(c == 0), stop=False)
                    nc.tensor.matmul(ps[:], ones_row[:, 0:128], brow[:],
                                     start=False, stop=True)
                    nc.scalar.copy(ysb[:, t, :], ps[:])

            yp_sb = pp_.tile([128, PT, H], f32, tag="yp_sb")
            yq_sb = pp_.tile([128, QT, H], f32, tag="yq_sb")
            project(p2own_fm, wp, bp_row, yp_sb, PT)
            project(q2own_fm, wq, bq_row, yq_sb, QT)
            nc.sync.dma_start(y_p[:].rearrange("(t p) h -> p t h", p=128), yp_sb[:])
            nc.scalar.dma_start(y_q[:].rearrange("(t p) h -> p t h", p=128), yq_sb[:])

            ps = psx.tile([QNH, H], f32, tag="psx")
            for c in range(2):
                nc.tensor.matmul(ps[:], qu2own_fm[:, c, :], wqu[:, c, :],
                                 start=(c == 0), stop=False)
            nc.tensor.matmul(ps[:], ones_row[:, 0:QNH], bqu_row[:],
                             start=False, stop=True)
            yqu_sb = stg.tile([QNH, H], f32, tag="yqu_sb")
            nc.scalar.copy(yqu_sb[:], ps[:])
            nc.sync.dma_start(y_qu[:], yqu_sb[:])

    _fix_multiwait(nc, mybir)
    return nc


def _host_prep(inputs):
    f32 = np.float32
    bf = ml_dtypes.bfloat16
    pm = inputs["p_node_mask"].astype(f32)
    qm = inputs["q_node_mask"].astype(f32)
    qum = inputs["question_node_mask"].astype(f32)
    gpp = inputs["pp_graph"]
    gqq = inputs["qq_graph"]

    nb_p = pm * np.einsum("bij,bj->bi", gpp.astype(f32), pm)
    nb_p = np.where(nb_p >= 1, nb_p, 1.0)
    npm = pm.sum(-1)
    nqm = qm.sum(-1)
    nb_q = qm * (npm[:, None] + np.einsum("bij,bj->bi", gqq.astype(f32), qm))
    nb_q = np.where(nb_q >= 1, nb_q, 1.0)
    nb_qu = qum * (npm[:, None] + nqm[:, None])
    nb_qu = np.where(nb_qu >= 1, nb_qu, 1.0)
    s_p = pm / nb_p
    s_q = qm / nb_q
    s_qu = qum / nb_qu

    W_node = inputs["W_node"].astype(f32)
    b_node = float(np.asarray(inputs["b_node"]).reshape(-1)[0])

    def two_chunk(w):  # [256, N] -> [128, 2*N] with [p, c*N+n] = w[c*128+p, n]
        n = w.shape[1]
        return w.reshape(2, 128, n).transpose(1, 0, 2).reshape(128, 2 * n)

    wcommon = np.zeros((128, WBF), f32)

    def put_bf(nm, w):
        off, width = _BF_SLICES[nm]
        wcommon[:, off:off + width] = w

    put_bf("wcat_p", two_chunk(
        np.concatenate([inputs["W_pp"].astype(f32), W_node], 1)))
    put_bf("wcat_q", two_chunk(
        np.concatenate([inputs["W_qq"].astype(f32), W_node], 1)))
    put_bf("wself", two_chunk(inputs["W_self"].astype(f32)))
    put_bf("wqd", two_chunk(inputs["W_qd"].astype(f32)))
    put_bf("wqup", two_chunk(inputs["W_qup"].astype(f32)))
    put_bf("wquq", two_chunk(inputs["W_quq"].astype(f32)))
    put_bf("wp", two_chunk(inputs["W_p"].astype(f32)))
    put_bf("wq", two_chunk(inputs["W_q"].astype(f32)))
    put_bf("wqu", two_chunk(inputs["W_qu"].astype(f32)))

    wf_common = np.zeros((128, WF32), f32)
    wf_common[:, 0:2] = inputs["b_self"].astype(f32).reshape(2, 128).T

    def cols(v):
        return np.ascontiguousarray(v.reshape(-1, 128).T)

    in_maps = []
    for k in range(NC_):
        b, h = divmod(k, 2)
        sp_ = slice(h * PH, (h + 1) * PH)
        sq_ = slice(h * QH, (h + 1) * QH)
        squ = slice(h * QNH, (h + 1) * QNH)
        p0b = inputs["p_node"][b].astype(f32)
        q0b = inputs["q_node"][b].astype(f32)

        wb = wcommon.copy()
        rows_off = _BF_SLICES["rows"][0]

        def put_row(nm, v, wb=wb, rows_off=rows_off):
            off, width = _ROWS[nm]
            wb[0, rows_off + off:rows_off + off + width] = v

        put_row("bp", inputs["b_p"].astype(f32))
        put_row("bq", inputs["b_q"].astype(f32))
        put_row("bqu", inputs["b_qu"].astype(f32))
        put_row("sqrow", s_q[b, sq_])
        put_row("squrow", s_qu[b, squ])
        put_row("ones", np.ones(128, f32))

        wf = wf_common.copy()

        def put_f32(nm, v, wf=wf):
            off, width = _F32_SLICES[nm]
            wf[:, off:off + width] = v

        put_f32("bmask_p", cols(b_node + (pm[b] - 1.0) * 30.0))
        put_f32("bmask_q", cols(b_node + (qm[b] - 1.0) * 30.0))
        put_f32("smp", np.broadcast_to(s_p[b, sp_], (128, PH)))
        put_f32("smq", np.broadcast_to(s_q[b, sq_], (128, QH)))

        nfm = np.zeros((128, 2, NFM), f32)

        def put_fm(nm, arrT, nfm=nfm):
            off, width = _FM_SLICES[nm]
            nfm[:, :, off:off + width] = arrT.reshape(2, 128, -1).transpose(1, 0, 2)

        put_fm("p0", p0b.T)
        put_fm("q0", q0b.T)
        put_fm("p0own", p0b[sp_].T)
        put_fm("q0own", q0b[sq_].T)
        put_fm("qu0own", inputs["question_node"][b][squ].astype(f32).T)

        ntm = np.concatenate([
            p0b.reshape(PJ, 128, D).transpose(1, 0, 2),
            q0b.reshape(QJ, 128, D).transpose(1, 0, 2),
        ], axis=1)

        m = dict(
            g_ppT=np.ascontiguousarray(gpp[b, sp_].T),
            g_qqT=np.ascontiguousarray(gqq[b, sq_].T),
            wblob_bf=wb.astype(bf),
            wblob_f32=wf,
            node_fm=nfm.astype(bf),
            node_tm=np.ascontiguousarray(ntm).astype(bf),
        )
        in_maps.append(m)
    return in_maps


def kernel(**inputs):
    from concourse.bass_utils import run_bass_kernel_spmd

    if "nc" not in _BUILD_CACHE:
        _BUILD_CACHE["nc"] = _build()
    nc = _BUILD_CACHE["nc"]
    in_maps = _host_prep(inputs)
    res = run_bass_kernel_spmd(nc, in_maps, core_ids=list(range(NC_)))
    r = res.results

    p_out = np.zeros((B, P, H), np.float32)
    q_out = np.zeros((B, Q, H), np.float32)
    qu_out = np.zeros((B, QN, H), np.float32)
    for k in range(NC_):
        b, h = divmod(k, 2)
        p_out[b, h * PH:(h + 1) * PH] = r[k]["y_p"]
        q_out[b, h * QH:(h + 1) * QH] = r[k]["y_q"]
        qu_out[b, h * QNH:(h + 1) * QNH] = r[k]["y_qu"]
    return p_out, q_out, qu_out
